# revision 23
# baseline (speedup 1.0000x reference)
"""GCNConv (X @ W, then unweighted CSR neighbor-sum) on 8 TRN2 NeuronCores.

Strategy (hardcoded for N=50000, E=800000, D_in=128, D_out=64, 8 cores):
  - Destination nodes are sharded: core k owns rows [6250k, 6250(k+1)).
    Edges follow their (sorted) destination row, so each core gets a
    contiguous slice of the edge list.  The weight matrix is replicated.
  - Phase 1 (replicated): every core computes the full transformed table
    X' = X @ W from a host-pretransposed X^T (bf16) and stores it in a
    DRAM scratch.  Replicating this beats an AllGather of shards (the
    collective path is fold_n-limited to ~54 GB/s per link).
  - Phase 2 (sharded): neighbor rows are fetched with indirect DMA in
    its only HW-correct form on this stack (probed): one int32 offset
    per partition per call, each partition receiving a contiguous run
    from the table.  One call gathers one 128-edge tile.  The segment
    sum is a collision-free one-hot matmul: per 64-node dest block,
    M[lane, dest] = (rowrel == iota), accumulated into PSUM as M^T @ G.
  - Host preprocessing is index manipulation + layout only (shard/sort/
    pad/transpose/cast); all FLOPs on tensor data happen on device.
"""

from contextlib import ExitStack

import numpy as np
import ml_dtypes

import concourse.bass as bass
import concourse.mybir as mybir
import concourse.tile as tile
from concourse import bacc
from concourse.bass_utils import run_bass_kernel_spmd

# ---- problem constants (must match the harness inputs) ----
N_NODES = 50000
N_EDGES = 800000
D_IN = 128
D_OUT = 64
N_CORES = 8

NODES_PER_CORE = N_NODES // N_CORES            # 6250
BLK = 64                                       # dest-block width (matmul M dim)
BLOCKS_PER_CORE = (NODES_PER_CORE + BLK - 1) // BLK   # 98
NODES_PAD_PER_CORE = BLOCKS_PER_CORE * BLK     # 6272
NODES_PAD = ((N_NODES + 1023) // 1024) * 1024  # 50176 = 49 * 1024
N_PAIRS = NODES_PAD // 2                       # 25088 (< int16 max)
GROUP = 8                                      # node tiles per phase-1 group
N_GROUPS = NODES_PAD // (128 * GROUP)          # 49
CHUNK = 7                                      # dest blocks per phase-2 gather
N_CHUNKS = BLOCKS_PER_CORE // CHUNK            # 14

ST_DT = mybir.dt.bfloat16                      # storage dtype for X^T / W / X'
NP_ST = ml_dtypes.bfloat16

# test.py can flip this to get a profiled run; results land in LAST_RESULTS.
TRACE = False
LAST_RESULTS = None


def _xp_perm_pos(r):
    """DRAM row position of node r in the permuted X' table.

    Phase 1 emits X' from SBUF tiles shaped [lane p, tile t, feat]; storing
    node (g*1024 + t*128 + p) at position (g*1024 + p*8 + t) makes each
    lane's 8 rows contiguous (1KB descriptors instead of 128B).
    """
    g = r >> 10
    rem = r & 1023
    t = rem >> 7
    p = rem & 127
    return (g << 10) + p * GROUP + t


def build_program(T: int):
    """One SPMD program shared by all 8 cores (per-core variation is data)."""
    NT = BLOCKS_PER_CORE * T                   # edge tiles per core

    nc = bacc.Bacc("TRN2", target_bir_lowering=False, debug=False,
                   num_devices=N_CORES)
    xt = nc.dram_tensor("xt", [D_IN, NODES_PAD], ST_DT,
                        kind="ExternalInput").ap()
    w = nc.dram_tensor("w", [D_IN, D_OUT], ST_DT, kind="ExternalInput").ap()
    cols = nc.dram_tensor("cols", [128, NT], mybir.dt.int32,
                          kind="ExternalInput").ap()
    rowrel = nc.dram_tensor("rowrel", [128, NT], ST_DT,
                            kind="ExternalInput").ap()
    iota = nc.dram_tensor("iota", [128, BLK], ST_DT,
                          kind="ExternalInput").ap()
    out = nc.dram_tensor("out", [NODES_PAD_PER_CORE, D_OUT],
                         mybir.dt.float32, kind="ExternalOutput").ap()
    # +2 pad rows: the bf16 indirect DMA fetches a run of 2 rows per offset
    xp = nc.dram_tensor("xp", [NODES_PAD + 2, D_OUT], ST_DT).ap()

    with tile.TileContext(nc) as tc:
        with (
            tc.tile_pool(name="const", bufs=1) as cpool,
            tc.tile_pool(name="xt", bufs=3) as xtpool,
            tc.tile_pool(name="xps", bufs=3) as xpool,
            tc.tile_pool(name="p1ps", bufs=2, space="PSUM") as p1psum,
            tc.tile_pool(name="gat", bufs=2) as gpool,
            tc.tile_pool(name="msel", bufs=2) as mpool,
            tc.tile_pool(name="p2ps", bufs=4, space="PSUM") as p2psum,
            tc.tile_pool(name="ob", bufs=4) as opool,
        ):
            # ---- constants ----
            w_sb = cpool.tile([D_IN, D_OUT], ST_DT)
            nc.sync.dma_start(w_sb[:], w[:])
            iota_sb = cpool.tile([128, BLK], ST_DT)
            nc.sync.dma_start(iota_sb[:], iota[:])
            cols_sb = cpool.tile([128, NT], mybir.dt.int32)
            nc.sync.dma_start(cols_sb[:], cols[:])
            rr_sb = cpool.tile([128, NT], ST_DT)
            nc.sync.dma_start(rr_sb[:], rowrel[:])
            zpad = cpool.tile([2, D_OUT], ST_DT)
            nc.gpsimd.memset(zpad[:], 0.0)
            nc.sync.dma_start(xp[NODES_PAD:NODES_PAD + 2, :], zpad[:])

            # ---- phase 1: X' = X @ W, stored permuted+paired in DRAM ----
            for g in range(N_GROUPS):
                n0 = g * 128 * GROUP
                xt_t = xtpool.tile([128, 128 * GROUP], ST_DT)
                nc.sync.dma_start(xt_t[:], xt[:, n0:n0 + 128 * GROUP])
                ps = p1psum.tile([128, GROUP * D_OUT], mybir.dt.float32)
                for t in range(GROUP):
                    nc.tensor.matmul(
                        out=ps[:, t * D_OUT:(t + 1) * D_OUT],
                        lhsT=xt_t[:, t * 128:(t + 1) * 128],
                        rhs=w_sb[:],
                        start=True, stop=True)
                xp_sb = xpool.tile([128, GROUP * D_OUT], ST_DT)
                nc.vector.tensor_copy(xp_sb[:], ps[:])
                # lane p holds nodes n0 + t*128 + p (t = 0..7) -> permuted
                # positions n0 + p*8 + t -> pair rows n0/2 + p*4 + (0..3).
                nc.sync.dma_start(
                    xp[g * 1024:(g + 1) * 1024, :].rearrange(
                        "(p t) c -> p t c", t=GROUP),
                    xp_sb[:].rearrange("p (t c) -> p t c", c=D_OUT))

            # ---- phase 2: per-tile indirect gather + one-hot matmul ----
            ntc = CHUNK * T                    # edge tiles per chunk
            for c in range(N_CHUNKS):
                t0 = c * ntc                   # first edge tile of chunk
                m_t = mpool.tile([128, ntc, BLK], ST_DT)
                nc.vector.tensor_tensor(
                    out=m_t[:],
                    in0=rr_sb[:, t0:t0 + ntc].unsqueeze(2).to_broadcast(
                        [128, ntc, BLK]),
                    in1=iota_sb[:].unsqueeze(1).to_broadcast(
                        [128, ntc, BLK]),
                    op=mybir.AluOpType.is_equal)
                # one indirect call per edge tile: one offset per partition,
                # each lane receives a contiguous 2-row bf16 run (row 0 is
                # the wanted source).  NOTE: the indirect DMA's out AP must
                # be rank-2 [128, elems] — higher ranks mis-lower on this
                # stack (probed: only partition 0 gets written).
                g_ch = gpool.tile([128, ntc, 2 * D_OUT], ST_DT)
                for ti in range(ntc):
                    nc.gpsimd.indirect_dma_start(
                        out=g_ch[:, ti, :],
                        out_offset=None,
                        in_=xp[:],
                        in_offset=bass.IndirectOffsetOnAxis(
                            ap=cols_sb[:, t0 + ti:t0 + ti + 1], axis=0))
                for b in range(CHUNK):
                    gb = c * CHUNK + b         # global block id on this core
                    ps2 = p2psum.tile([BLK, D_OUT], mybir.dt.float32)
                    for t in range(T):
                        ti = b * T + t
                        nc.tensor.matmul(
                            out=ps2[:],
                            lhsT=m_t[:, ti, :],
                            rhs=g_ch[:, ti, 0:D_OUT],
                            start=(t == 0), stop=(t == T - 1))
                    ob = opool.tile([BLK, D_OUT], mybir.dt.float32)
                    nc.vector.tensor_copy(ob[:], ps2[:])
                    nc.sync.dma_start(out[gb * BLK:(gb + 1) * BLK, :], ob[:])

    nc.compile()
    return nc


def prepare_inputs(X, weights, row_index, column_index):
    """Host-side shard/pad/layout. Returns (T, in_maps)."""
    row = np.ascontiguousarray(row_index).astype(np.int64)
    col = np.ascontiguousarray(column_index).astype(np.int64)

    core_bounds = np.searchsorted(
        row, np.arange(N_CORES + 1) * NODES_PER_CORE)

    # per-core local block boundaries
    per_core = []
    max_cnt = 1
    for k in range(N_CORES):
        lo, hi = core_bounds[k], core_bounds[k + 1]
        r = row[lo:hi] - k * NODES_PER_CORE          # local rows [0, 6250)
        c = col[lo:hi]
        bb = np.searchsorted(r, np.arange(BLOCKS_PER_CORE + 1) * BLK)
        cnts = np.diff(bb)
        if len(cnts) and cnts.max() > max_cnt:
            max_cnt = int(cnts.max())
        per_core.append((r, c, bb))
    T = (max_cnt + 127) // 128
    NT = BLOCKS_PER_CORE * T
    NI = NT * 128

    # shared tensors
    xt_full = np.zeros((D_IN, NODES_PAD), dtype=NP_ST)
    xt_full[:, :N_NODES] = np.ascontiguousarray(X.T).astype(NP_ST)
    w_np = np.ascontiguousarray(weights).astype(NP_ST)
    iota_np = np.broadcast_to(
        np.arange(BLK, dtype=np.float32), (128, BLK)).astype(NP_ST)

    in_maps = []
    for k in range(N_CORES):
        r, c, bb = per_core[k]
        b = r >> 6                                   # local block per edge
        idx = np.arange(len(r))
        slot = b * (T * 128) + (idx - bb[b])         # slot within core
        cols_flat = np.zeros(NI, dtype=np.int64)     # pad slots gather row 0
        cols_flat[slot] = _xp_perm_pos(c)            # permuted X' row
        rr = np.full(NI, -1.0, dtype=np.float32)
        rr[slot] = (r - b * BLK).astype(np.float32)  # 0..63
        in_maps.append({
            "xt": xt_full,
            "w": w_np,
            "iota": iota_np,
            "cols": np.ascontiguousarray(
                cols_flat.reshape(NT, 128).T).astype(np.int32),
            "rowrel": np.ascontiguousarray(
                rr.reshape(NT, 128).T).astype(NP_ST),
        })
    return T, in_maps


def kernel(X, weights, row_index, column_index):
    global LAST_RESULTS
    T, in_maps = prepare_inputs(X, weights, row_index, column_index)
    nc = build_program(T)
    res = run_bass_kernel_spmd(nc, in_maps, list(range(N_CORES)),
                               trace=TRACE)
    LAST_RESULTS = res
    out = np.concatenate(
        [res.results[k]["out"][:NODES_PER_CORE] for k in range(N_CORES)],
        axis=0)
    return out.astype(np.float32)
